# revision 47
# baseline (speedup 1.0000x reference)
"""Trainium2 Bass kernel for nn_LocatorReaderConditioner (cross-attention block).

Reference computation (per batch b):
    q = query @ Wq, k = mem @ Wk, v = mem @ Wv   (split into 16 heads of 64)
    scores = q k^T / sqrt(64) + bias[None, :]
    out = softmax(scores) v   (concat heads)  @ Wo

Sharding over 8 cores: core c handles batch b = c // 2 and head-group
hg = c % 2 (8 heads, 512 feature columns of Wq/Wk/Wv, 512 rows of Wo).
Each core returns a partial output (its head-group pushed through its Wo
rows); the host sums the two partials per batch (the "all-reduce").

Layout: mem^T and qry^T are packed on the HOST, so device loads are
plain max-rate DMAs (no XBAR transposes). All math is bf16 (fp8
anywhere in the QKV/softmax path exceeds the accuracy budget).

Schedule: one flat software-pipelined stream over all 128 (pair, ktile)
steps. The V projection and each pair's K^T projection are emitted
just-in-time inside the stream so both hide inside attention ktiles.
Scores are computed transposed ([ktok, q]) and row-packed per head pair
(concurrent K=64 matmuls on array rows 0-63/64-127); the exp is one
fused ACT per ktile; the softmax denominator rides a ones-column in
each head's V block. AV flushes lag the exp stream by 2 positions.

DMA plan: the early fill is HBM-bandwidth-bound (~6.5MB must land in
the first ~20us), so every transfer is a flat per-partition-contiguous
slice (128 descriptors of 1-8KB: ~5x cheaper DIRECT2D trigger gen and
full line rate), each mem chunk is split in half across the
sync+gpsimd queues, and all three queues are sequenced in global need
order (qry/wq interleaved in Q-projection consumption order ->
mem0/wk_m0 -> wv/bias -> mem1..7 -> wk_rest -> wo).  The scalar queue
carries only pre-exp data (its sequencer runs the exp stream; a DMA
trigger enqueued behind a pending transfer would stall the exps), and
nothing first-needed (it starts ~3us late behind ACT_TABLE_LOAD).
kproj JIT production is shifted 2 positions later than consumption
pace requires (group n at g=4n-6..4n-3, consumed at 4n) so it
tolerates just-in-time chunk arrival; vproj(k) runs at g=k+1.

Steady-state pace in pairs 1-3 is ~1.29us/position with BOTH the PE
(4 full matmuls + row-packed scores pair) and the ACT engine (one
[128,1024] exp + inter-instruction bubble) saturated — a dual-engine
floor.  A dummy partition_broadcast during the fill preloads the
gpsimd ucode library (first use otherwise costs ~8us mid-stream and
convoys the DVE FIFO, the kT evacuations, and then the in-order PE).
p_pool holds 5 exp outputs so the exp stream never waits on AV
consumption; normalize stage copies are emitted for both heads before
any per-head tail work, and the recip/broadcast/mul phase is deferred
3 positions (pairs 0-2) to keep kT evacuations unblocked on the DVE.

The output partial is stored partition-major ([128, 4, 1024] DRAM,
host reassembles) in two flat 4KB-per-partition stores on the
scalar+sync queues; normalize partition hops run on HWDGE queues
(pair 3's hops split across scalar and sync to avoid FIFO chaining in
the tail).  Wo phase 1 runs att-stationary-outer so each att_pairs
LDWEIGHTS serves two matmuls; phase 2 is qt-outer for the same reason,
with psum drained by alternating vector/scalar casts into one
[128, 4, 1024] tile.
"""

from contextlib import ExitStack

import numpy as np
import ml_dtypes

import bass_rust
import concourse.bass as bass
import concourse.mybir as mybir
import concourse.tile as tile
from concourse import bacc
from concourse.bass import ds, ts
from concourse.bass_utils import run_bass_kernel_spmd

BF16 = ml_dtypes.bfloat16
F32 = np.float32

B, Q, KT, D = 4, 512, 4096, 1024
H_PER_CORE = 8          # heads per core
DH = 64                 # head dim
DG = 512                # feature columns per core (H_PER_CORE * DH)
SCALE = DH ** -0.5
N_CORES = 8
KTILES = KT // 128      # 32
KCHUNK = 8              # din tiles (D / 128)
NCH = 8                 # ktok chunks of 512
QT = 4                  # q row-blocks of 128

_CACHE: dict = {}


def _build_nc():
    nc = bacc.Bacc("TRN2", target_bir_lowering=False, debug=False)
    dt = mybir.dt

    memT = nc.dram_tensor("memT", [128, NCH, KCHUNK * 512], dt.bfloat16,
                          kind="ExternalInput")
    qryT = nc.dram_tensor("qryT", [128, KCHUNK, Q], dt.bfloat16,
                          kind="ExternalInput")
    # wq/wk packed dout-tile-major: [p, mt, kt, j] = W[kt*128+p, mt*128+j]
    wq4 = nc.dram_tensor("wq4", [128, 4, KCHUNK, 128], dt.bfloat16,
                         kind="ExternalInput")
    wk4 = nc.dram_tensor("wk4", [128, 4, KCHUNK, 128], dt.bfloat16,
                         kind="ExternalInput")
    wv = nc.dram_tensor("wv", [128, KCHUNK, DG], dt.bfloat16, kind="ExternalInput")
    wo = nc.dram_tensor("wo", [128, 4, D], dt.bfloat16, kind="ExternalInput")
    biasT = nc.dram_tensor("biasT", [128, KTILES], dt.float32, kind="ExternalInput")
    # partition-major output: out[p, qt, d] = partial[qt*128 + p, d]
    out = nc.dram_tensor("out", [128, QT, D], dt.bfloat16, kind="ExternalOutput")

    with tile.TileContext(nc) as tc, ExitStack() as ctx:
        const = ctx.enter_context(tc.tile_pool(name="const", bufs=1))

        mem_tiles = [
            const.tile([128, KCHUNK, 512], dt.bfloat16, name=f"mem_{c}")
            for c in range(NCH)
        ]
        kT = const.tile([128, 4, KT], dt.bfloat16)             # K^T  (dout, ktok)
        qT = const.tile([128, 4, Q], dt.bfloat16)              # Q^T  (dout, q)
        early_cm = tc.tile_pool(name="early", bufs=1)
        early = early_cm.__enter__()
        qry_q = [early.tile([128, 2, Q], dt.bfloat16, name=f"qry_{i}")
                 for i in range(4)]
        wq_mt = [early.tile([128, KCHUNK, 128], dt.bfloat16, name=f"wqm_{i}")
                 for i in range(4)]
        wv_h = [const.tile([128, 4, DG], dt.bfloat16, name=f"wv_{i}")
                for i in range(2)]
        wk_m0 = const.tile([128, KCHUNK, 128], dt.bfloat16)
        wk_rest = const.tile([128, 3, KCHUNK, 128], dt.bfloat16)
        wo_s = const.tile([128, 4, D], dt.bfloat16)
        bias_s = const.tile([128, KTILES], dt.float32)
        v_tiles = [
            const.tile([128, H_PER_CORE * 65], dt.bfloat16, name=f"v_{k}")
            for k in range(KTILES)
        ]
        att_pairs = [
            const.tile([128, Q], dt.bfloat16, name=f"attp_{p}") for p in range(4)
        ]
        warm_w = const.tile([128, 128], dt.bfloat16)
        warm_x = const.tile([128, 512], dt.bfloat16)
        nc.vector.memset(warm_w, 0.0)
        nc.vector.memset(warm_x, 0.0)
        warm_bc_in = const.tile([1, 64], dt.float32)
        warm_bc = const.tile([64, 64], dt.float32)

        # ---- DMA queue plan: flat slices, global need order, 3 queues.
        # scalar must clear before the first exp; sync/gpsimd split the
        # mem chunks in kt halves (kt 0-3 | kt 4-7).
        flat = lambda ap: ap.rearrange("p a b -> p (a b)")
        mem_f = lambda c: memT.ap()[:, c]                     # [128, 4096]
        mem_t = lambda c: mem_tiles[c].rearrange("p k t -> p (k t)")

        # scalar (slow start: ACT_TABLE_LOAD precedes its first trigger, so
        # nothing first-needed goes here): wq mt2, bias, wv half1
        nc.scalar.dma_start(wq_mt[2], wq4.ap()[:, 2])
        nc.scalar.dma_start(bias_s, biasT.ap())
        nc.scalar.dma_start(wv_h[1], wv.ap()[:, 4:8])
        # sync: qry/wq in consumption order, quarter-size pieces (each
        # trigger costs ~0.6us serial descriptor-gen on the queue)
        nc.sync.dma_start(qry_q[0], qryT.ap()[:, 0:2])
        nc.sync.dma_start(wq_mt[0], wq4.ap()[:, 0])
        nc.sync.dma_start(qry_q[1], qryT.ap()[:, 2:4])
        nc.sync.dma_start(wq_mt[3], wq4.ap()[:, 3])
        nc.sync.dma_start(mem_t(0)[:, 0:2048], mem_f(0)[:, 0:2048])
        nc.sync.dma_start(wv_h[0], wv.ap()[:, 0:4])
        nc.sync.dma_start(mem_t(1)[:, 2048:4096], mem_f(1)[:, 2048:4096])
        for c in (2, 3, 4, 5, 6, 7):
            nc.sync.dma_start(mem_t(c)[:, 0:2048], mem_f(c)[:, 0:2048])
        nc.sync.dma_start(wk_rest.rearrange("p m k j -> p (m k j)"),
                          wk4.ap()[:, 1:4].rearrange("p m k j -> p (m k j)"))
        # gpsimd: qry q2/q3, wq mt1, wk_m0, mem0 kt4-7, mem1a, ..., wo
        nc.gpsimd.dma_start(qry_q[2], qryT.ap()[:, 4:6])
        nc.gpsimd.dma_start(wq_mt[1], wq4.ap()[:, 1])
        nc.gpsimd.dma_start(qry_q[3], qryT.ap()[:, 6:8])
        nc.gpsimd.dma_start(wk_m0, wk4.ap()[:, 0])
        nc.gpsimd.dma_start(mem_t(0)[:, 2048:4096], mem_f(0)[:, 2048:4096])
        nc.gpsimd.dma_start(mem_t(1)[:, 0:2048], mem_f(1)[:, 0:2048])
        for c in (2, 3, 4, 5, 6, 7):
            nc.gpsimd.dma_start(mem_t(c)[:, 2048:4096], mem_f(c)[:, 2048:4096])
        nc.gpsimd.dma_start(wo_s.rearrange("p a b -> p (a b)"),
                            flat(wo.ap()))
        # warm the gpsimd partition_broadcast path (ucode library load +
        # first-use latency is ~8us; pair 0's normalize hits it otherwise).
        # Emitted AFTER the gpsimd dma triggers so the library load doesn't
        # delay SWDGE descriptor generation for the critical input fill.
        nc.vector.memset(warm_bc_in, 1.0)
        nc.gpsimd.partition_broadcast(warm_bc, warm_bc_in)

        for k in range(KTILES):
            vb = v_tiles[k].rearrange("p (h c) -> p h c", c=65)
            nc.vector.memset(vb[:, :, 64:65], 1.0)

        ps_stack = ExitStack()

        # HAM warmup: dependency-free matmuls keep the PE busy while DMAs
        # fill SBUF so the clock-gate is at full rate when real work starts.
        with tc.tile_pool(name="warm_ps", bufs=1, space="PSUM") as warm_ps:
            wps = warm_ps.tile([128, 512], dt.float32, tag="warm")
            for i in range(10):
                nc.tensor.matmul(wps, warm_w, warm_x,
                                 start=(i == 0), stop=(i == 9))

        proj_ps = ctx.enter_context(
            tc.tile_pool(name="proj_ps", bufs=2, space="PSUM")
        )

        kproj_state = {}

        def kproj_mms(mt, c, kts):
            """Emit K^T-projection matmuls for dout tile mt, chunk c, din
            tiles `kts`; evacuate after the last."""
            if mt == 0:
                wsrc, col = wk_m0, slice(None)
            else:
                wsrc, col = wk_rest[:, mt - 1], slice(None)
            if kts[0] == 0:
                kproj_state[(mt, c)] = proj_ps.tile(
                    [128, 512], dt.float32, tag="proj",
                    name=f"kproj_ps_{mt}_{c}")
            ps = kproj_state[(mt, c)]
            for kt in kts:
                nc.tensor.matmul(
                    ps, wsrc[:, kt, col], mem_tiles[c][:, kt, :],
                    start=(kt == 0), stop=(kt == KCHUNK - 1),
                )
            if kts[-1] == KCHUNK - 1:
                nc.vector.tensor_copy(kT[:, mt, ts(c, 512)], ps)
                del kproj_state[(mt, c)]

        def vproj(k):
            """V natural rows for ktile k (8 MMs + evac into v_tiles[k])."""
            c, j = k // 4, k % 4
            ps = proj_ps.tile([128, DG], dt.float32, tag="proj")
            for kt in range(KCHUNK):
                nc.tensor.matmul(
                    ps, mem_tiles[c][:, kt, ds(j * 128, 128)],
                    wv_h[kt // 4][:, kt % 4, :],
                    start=(kt == 0), stop=(kt == KCHUNK - 1),
                )
            vb = v_tiles[k].rearrange("p (h c) -> p h c", c=65)
            nc.vector.tensor_copy(
                vb[:, :, 0:64], ps.rearrange("p (h c) -> p h c", c=64)
            )

        # ---- Q projection: qT[dout, q] ----
        for mt in range(4):
            ps = proj_ps.tile([128, Q], dt.float32, tag="proj")
            for kt in range(KCHUNK):
                nc.tensor.matmul(
                    ps, wq_mt[mt][:, kt, :],
                    qry_q[kt // 2][:, kt % 2, :],
                    start=(kt == 0), stop=(kt == KCHUNK - 1),
                )
            nc.vector.tensor_copy(qT[:, mt, :], ps)
        early_cm.__exit__(None, None, None)

        # kT group (0,0) consumed at g=0; emit before the stream starts.
        kproj_mms(0, 0, list(range(KCHUNK)))

        # ---- flat attention stream over (pair, ktile) ----
        scores_ps = ps_stack.enter_context(
            tc.tile_pool(name="scores_ps", bufs=2, space="PSUM"))
        av_ps = ps_stack.enter_context(
            tc.tile_pool(name="av_ps", bufs=1, space="PSUM"))
        # bufs=5: with 3, exp(g) waits on AV matmuls of g-3 (+sem hop) and
        # the exp stream idles ~200ns/position in pairs 1-3
        p_pool = ps_stack.enter_context(tc.tile_pool(name="p_pool", bufs=5))
        norm_pool = ps_stack.enter_context(tc.tile_pool(name="norm_pool", bufs=1))

        avs = {}
        pending = []
        LAG = 2
        wo_groups = []

        def flush_one():
            pair, k, p_sb = pending.pop(0)
            if k == 0:
                avs[pair] = (
                    av_ps.tile([65, Q], dt.float32, tag="av0", name=f"av0_{pair}"),
                    av_ps.tile([65, Q], dt.float32, tag="av1", name=f"av1_{pair}"),
                )
            av = avs[pair]
            vb = v_tiles[k].rearrange("p (h c) -> p h c", c=65)
            for par, h in ((0, 2 * pair), (1, 2 * pair + 1)):
                nc.tensor.matmul(
                    av[par], vb[:, h, :], p_sb[:, par * Q:(par + 1) * Q],
                    start=(k == 0), stop=(k == KTILES - 1),
                )
            if k == KTILES - 1:
                pend = norm_phase1(pair)
                if pair == 3:
                    norm_phase2(pair, *pend)
                else:
                    norm_pend[pair] = pend

        def norm_phase1(pair):
            """Evacuate av psum + start the denominator hops.

            ALL four stage copies are emitted first so the av psum releases
            after ~1us of DVE work.  The denominator row (partition 64) is
            DMA-hopped to partition 0 for partition_broadcast (split across
            two queues: sync+gpsimd mid-stream, scalar+sync for pair 3).
            """
            av = avs[pair]
            den_eng = {1: nc.scalar if pair == 3 else nc.sync,
                       0: nc.sync if pair == 3 else nc.gpsimd}
            # the tiny denominator-row copies go first (so the den hops and
            # the reciprocal chain start ~1us earlier), then the two big
            # head copies; DMA can't read psum directly.
            stages = {}
            for par, h in ((1, 2 * pair + 1), (0, 2 * pair)):
                stage = norm_pool.tile([65, Q], dt.float32, tag="stage", bufs=2,
                                       name=f"stage_{h}")
                nc.vector.tensor_copy(stage[64:65, :], av[par][64:65, :])
                stages[par] = stage
            dens = {}
            for par, h in ((1, 2 * pair + 1), (0, 2 * pair)):
                den = norm_pool.tile([1, Q], dt.float32, tag="den", bufs=2,
                                     name=f"den_{h}")
                den_eng[par].dma_start(den, stages[par][64:65, :])
                dens[par] = den
            for par in (1, 0):
                nc.vector.tensor_copy(stages[par][0:64, :], avs[pair][par][0:64, :])
            return stages, dens

        def norm_phase2(pair, stages, dens):
            """Reciprocal + broadcast + normalize muls + odd-head hop.
            Deferred a few positions after phase 1 (for pairs 0-2) so the
            muls don't block kT evacuations in the DVE FIFO at the pair
            boundary; the odd head's normalized rows are DMA-hopped to
            partitions 64-127."""
            recvs = {}
            for par, h in ((1, 2 * pair + 1), (0, 2 * pair)):
                recv = norm_pool.tile([1, Q], dt.float32, tag="recv", bufs=2,
                                      name=f"recv_{h}")
                nc.vector.reciprocal_approx_fast(out=recv, in_=dens[par])
                recvs[par] = recv
            for par, h in ((1, 2 * pair + 1), (0, 2 * pair)):
                rec_bc = norm_pool.tile([64, Q], dt.float32, tag="rec_bc", bufs=2,
                                        name=f"rec_bc_{h}")
                nc.gpsimd.partition_broadcast(rec_bc, recvs[par])
                if par == 0:
                    nc.vector.tensor_mul(att_pairs[pair][0:64, :],
                                         stages[par][0:64, :], rec_bc)
                else:
                    odd = norm_pool.tile([64, Q], dt.bfloat16, tag="odd", bufs=2,
                                         name=f"odd_{h}")
                    nc.vector.tensor_mul(odd, stages[par][0:64, :], rec_bc)
                    eng = nc.scalar if pair == 3 else nc.sync
                    eng.dma_start(att_pairs[pair][64:128, :], odd)

        norm_pend = {}

        def wo_phase1(ps_pool, qt_i, dt_i, tag=None):
            ps = ps_pool.tile([128, 512], mybir.dt.float32,
                              tag=tag or f"wo_{qt_i}_{dt_i}",
                              name=f"wo_ps_{qt_i}_{dt_i}")
            for p in range(3):
                nc.tensor.matmul(
                    ps, att_pairs[p][:, ts(qt_i, 128)], wo_s[:, p, ts(dt_i, 512)],
                    start=(p == 0), stop=False,
                )
            wo_groups.append((qt_i, dt_i, ps))

        def scores(g):
            """Row-packed score pair for flat position g: even head on array
            rows 0-63, odd on 64-127 (concurrent K=64 matmuls)."""
            pair, k = g // 32, g % 32
            sc = scores_ps.tile([128, 2 * Q], dt.float32, tag="sc",
                                name=f"sc_{g}")
            nc.tensor.matmul(
                sc[:, 0:Q], kT[0:64, pair, ts(k, 128)], qT[0:64, pair, :],
                start=True, stop=True,
            )
            nc.tensor.matmul(
                sc[:, Q:2 * Q], kT[64:128, pair, ts(k, 128)], qT[64:128, pair, :],
                start=True, stop=True,
            )
            return sc

        # JIT production schedule: group n=1 at g=1,2 (4 MMs each); group
        # n>=2 spread 2 MMs/position over g=4n-6..4n-3 (consumed at g=4n).
        jit: dict[int, list] = {}
        jit.setdefault(1, []).append((0, 1, [0, 1, 2, 3]))
        jit.setdefault(2, []).append((0, 1, [4, 5, 6, 7]))
        for n in range(2, 32):
            for r in range(4):
                jit.setdefault(4 * n - 6 + r, []).append(
                    (n // 8, n % 8, [2 * r, 2 * r + 1]))

        sc_ahead = scores(0)
        for g in range(128):
            pair, k = g // 32, g % 32
            sc, sc_ahead = sc_ahead, (scores(g + 1) if g + 1 < 128 else None)
            p_sb = p_pool.tile([128, 2 * Q], dt.bfloat16, tag="p")
            nc.scalar.activation(
                p_sb, sc, mybir.ActivationFunctionType.Exp,
                bias=bias_s[:, k:k + 1], scale=SCALE,
            )
            pending.append((pair, k, p_sb))
            if pair == 0 and k >= 1:
                vproj(k - 1)
            elif g == 32:
                vproj(31)
            for mt, c, kts in jit.get(g, ()):
                kproj_mms(mt, c, kts)
            while len(pending) > LAG:
                flush_one()
            if g >= 36 and (g - 36) % 32 == 0 and (g - 36) // 32 in norm_pend:
                norm_phase2((g - 36) // 32, *norm_pend.pop((g - 36) // 32))
            # the K-proj JIT stream (last group done at g=121) frees proj_ps:
            # start two Wo psum groups in its banks
            if g == 122:
                wo_phase1(proj_ps, 0, 0, tag="proj")
            if g == 124:
                wo_phase1(proj_ps, 0, 1, tag="proj")
        while pending:
            flush_one()
        ps_stack.close()

        # remaining Wo groups: phase 1 (pairs 0-2) stationary-outer while
        # pair 3's normalize chain completes; phase 2 adds pair 3, casts
        # drain psum on two engines, two flat partition-major stores.
        with tc.tile_pool(name="wo_ps", bufs=1, space="PSUM") as wo_ps, \
                tc.tile_pool(name="out_pool", bufs=1) as out_pool:
            o_all = out_pool.tile([128, QT, D], mybir.dt.bfloat16, tag="o")
            rest = [(qt_i, dt_i) for qt_i in range(1, 4) for dt_i in range(2)]
            ps_map = {}
            for qt_i, dt_i in rest:
                ps_map[(qt_i, dt_i)] = wo_ps.tile(
                    [128, 512], mybir.dt.float32, tag=f"wo_{qt_i}_{dt_i}",
                    name=f"wo_ps_{qt_i}_{dt_i}")
            for p in range(3):
                for qt_i, dt_i in rest:
                    nc.tensor.matmul(
                        ps_map[(qt_i, dt_i)], att_pairs[p][:, ts(qt_i, 128)],
                        wo_s[:, p, ts(dt_i, 512)],
                        start=(p == 0), stop=False,
                    )
            for qt_i, dt_i in rest:
                wo_groups.append((qt_i, dt_i, ps_map[(qt_i, dt_i)]))
            # phase 2: qt-outer so att_pairs[3] stationaries serve 2 MMs
            groups_by_qt = {}
            for qt_i, dt_i, ps in wo_groups:
                groups_by_qt.setdefault(qt_i, []).append((dt_i, ps))
            for qt_i in range(4):
                for dt_i, ps in sorted(groups_by_qt[qt_i]):
                    nc.tensor.matmul(
                        ps, att_pairs[3][:, ts(qt_i, 128)],
                        wo_s[:, 3, ts(dt_i, 512)],
                        start=False, stop=True,
                    )
                for dt_i, ps in sorted(groups_by_qt[qt_i]):
                    if dt_i == 0:
                        nc.vector.tensor_copy(
                            o_all[:, qt_i, ds(0, 512)], ps)
                    else:
                        nc.scalar.copy(o_all[:, qt_i, ds(512, 512)], ps)
                if qt_i == 1:
                    nc.scalar.dma_start(
                        out.ap()[:, 0:2].rearrange("p a b -> p (a b)"),
                        o_all[:, 0:2].rearrange("p a b -> p (a b)"))
                if qt_i == 3:
                    nc.sync.dma_start(
                        out.ap()[:, 2:4].rearrange("p a b -> p (a b)"),
                        o_all[:, 2:4].rearrange("p a b -> p (a b)"))

    nc.compile()
    return nc


def _shard_inputs(query, memory, bias, Wq, Wk, Wv, Wo):
    """Host-side sharding + layout packing (per-core input dicts)."""
    in_maps = []
    packed = {}
    for hg in range(2):
        cols = slice(hg * DG, (hg + 1) * DG)
        # dout-tile-major packing: w4[p, mt, kt, j] = W[kt*128+p, mt*128+j]
        def mt_major(W):
            return np.ascontiguousarray(
                W[:, cols].reshape(KCHUNK, 128, 4, 128).transpose(1, 2, 0, 3)
            ).astype(BF16)
        packed[hg] = {
            "wq4": mt_major(Wq),
            "wk4": mt_major(Wk),
            "wv": np.ascontiguousarray(
                Wv[:, cols].reshape(KCHUNK, 128, DG).transpose(1, 0, 2)
            ).astype(BF16),
            # pair-stacked Wo: rows 0-63 = even head, 64-127 = odd head
            "wo": np.ascontiguousarray(
                Wo[cols, :].reshape(4, 128, D).transpose(1, 0, 2)
            ).astype(BF16),
        }
    for core in range(N_CORES):
        b, hg = core // 2, core % 2
        # memT[p, c, kt, t'] = mem[c*512 + t', kt*128 + p]
        m = memory[b].reshape(NCH, 512, KCHUNK, 128)        # (c, t', kt, p)
        memT_np = np.ascontiguousarray(m.transpose(3, 0, 2, 1)).astype(BF16)
        # qryT[p, kt, t] = query[t, kt*128 + p]
        qv = query[b].reshape(Q, KCHUNK, 128)               # (t, kt, p)
        qryT_np = np.ascontiguousarray(qv.transpose(2, 1, 0)).astype(BF16)
        in_maps.append(
            {
                "memT": memT_np.reshape(128, NCH, KCHUNK * 512),
                "qryT": qryT_np,
                "biasT": np.ascontiguousarray(bias[b].reshape(KTILES, 128).T).astype(F32),
                **packed[hg],
            }
        )
    return in_maps


def _get_nc():
    if "nc" not in _CACHE:
        _CACHE["nc"] = _build_nc()
    return _CACHE["nc"]


def run_sharded(inputs: dict, **run_kwargs):
    """Shard, run on 8 cores, gather. Returns (output, BassKernelResults)."""
    nc = _get_nc()
    in_maps = _shard_inputs(
        inputs["query"], inputs["memory"], inputs["bias"],
        inputs["Wq"], inputs["Wk"], inputs["Wv"], inputs["Wo"],
    )
    res = run_bass_kernel_spmd(nc, in_maps, core_ids=list(range(N_CORES)), **run_kwargs)
    out = np.empty((B, Q, D), dtype=F32)
    for b in range(B):
        # device output is partition-major [128, QT, D]
        o0 = res.results[2 * b]["out"].astype(F32)
        o1 = res.results[2 * b + 1]["out"].astype(F32)
        out[b] = (o0 + o1).transpose(1, 0, 2).reshape(Q, D)
    return out, res


def kernel(**inputs) -> np.ndarray:
    inputs = {k: np.asarray(v) for k, v in inputs.items()}
    out, _ = run_sharded(inputs)
    return out


# revision 48
# speedup vs baseline: 1.0249x; 1.0249x over previous
"""Trainium2 Bass kernel for nn_LocatorReaderConditioner (cross-attention block).

Reference computation (per batch b):
    q = query @ Wq, k = mem @ Wk, v = mem @ Wv   (split into 16 heads of 64)
    scores = q k^T / sqrt(64) + bias[None, :]
    out = softmax(scores) v   (concat heads)  @ Wo

Sharding over 8 cores: core c handles batch b = c // 2 and head-group
hg = c % 2 (8 heads, 512 feature columns of Wq/Wk/Wv, 512 rows of Wo).
Each core returns a partial output (its head-group pushed through its Wo
rows); the host sums the two partials per batch (the "all-reduce").

Layout: mem^T and qry^T are packed on the HOST, so device loads are
plain max-rate DMAs (no XBAR transposes). All math is bf16 (fp8
anywhere in the QKV/softmax path exceeds the accuracy budget).

Schedule: one flat software-pipelined stream over all 128 (pair, ktile)
steps. The V projection and each pair's K^T projection are emitted
just-in-time inside the stream so both hide inside attention ktiles.
Scores are computed transposed ([ktok, q]) and row-packed per head pair
(concurrent K=64 matmuls on array rows 0-63/64-127); the exp is one
fused ACT per ktile; the softmax denominator rides a ones-column in
each head's V block. AV flushes lag the exp stream by 2 positions.

DMA plan: the early fill is HBM-bandwidth-bound (~6.5MB must land in
the first ~20us), so every transfer is a flat per-partition-contiguous
slice (128 descriptors of 1-8KB: ~5x cheaper DIRECT2D trigger gen and
full line rate), each mem chunk is split in half across the
sync+gpsimd queues, and all three queues are sequenced in global need
order (qry/wq interleaved in Q-projection consumption order ->
mem0/wk_m0 -> wv/bias -> mem1..7 -> wk_rest -> wo).  The scalar queue
carries only pre-exp data (its sequencer runs the exp stream; a DMA
trigger enqueued behind a pending transfer would stall the exps), and
nothing first-needed (it starts ~3us late behind ACT_TABLE_LOAD).
kproj JIT production is shifted 2 positions later than consumption
pace requires (group n at g=4n-6..4n-3, consumed at 4n) so it
tolerates just-in-time chunk arrival; vproj(k) runs at g=k+1.

Steady-state pace in pairs 1-3 is ~1.29us/position with BOTH the PE
(4 full matmuls + row-packed scores pair) and the ACT engine (one
[128,1024] exp + inter-instruction bubble) saturated — a dual-engine
floor.  A dummy partition_broadcast during the fill preloads the
gpsimd ucode library (first use otherwise costs ~8us mid-stream and
convoys the DVE FIFO, the kT evacuations, and then the in-order PE).
p_pool holds 5 exp outputs so the exp stream never waits on AV
consumption; normalize stage copies are emitted for both heads before
any per-head tail work, and the recip/broadcast/mul phase is deferred
3 positions (pairs 0-2) to keep kT evacuations unblocked on the DVE.

The output partial is stored partition-major ([128, 4, 1024] DRAM,
host reassembles) in two flat 4KB-per-partition stores on the
scalar+sync queues; normalize partition hops run on HWDGE queues
(pair 3's hops split across scalar and sync to avoid FIFO chaining in
the tail).  Wo phase 1 runs att-stationary-outer so each att_pairs
LDWEIGHTS serves two matmuls; phase 2 is qt-outer for the same reason,
with psum drained by alternating vector/scalar casts into one
[128, 4, 1024] tile.
"""

from contextlib import ExitStack

import numpy as np
import ml_dtypes

import bass_rust
import concourse.bass as bass
import concourse.mybir as mybir
import concourse.tile as tile
from concourse import bacc
from concourse.bass import ds, ts
from concourse.bass_utils import run_bass_kernel_spmd

BF16 = ml_dtypes.bfloat16
F32 = np.float32

B, Q, KT, D = 4, 512, 4096, 1024
H_PER_CORE = 8          # heads per core
DH = 64                 # head dim
DG = 512                # feature columns per core (H_PER_CORE * DH)
SCALE = DH ** -0.5
N_CORES = 8
KTILES = KT // 128      # 32
KCHUNK = 8              # din tiles (D / 128)
NCH = 8                 # ktok chunks of 512
QT = 4                  # q row-blocks of 128

_CACHE: dict = {}


def _build_nc():
    nc = bacc.Bacc("TRN2", target_bir_lowering=False, debug=False)
    dt = mybir.dt

    memT = nc.dram_tensor("memT", [128, NCH, KCHUNK * 512], dt.bfloat16,
                          kind="ExternalInput")
    qryT = nc.dram_tensor("qryT", [128, KCHUNK, Q], dt.bfloat16,
                          kind="ExternalInput")
    # wq/wk packed dout-tile-major: [p, mt, kt, j] = W[kt*128+p, mt*128+j]
    wq4 = nc.dram_tensor("wq4", [128, 4, KCHUNK, 128], dt.bfloat16,
                         kind="ExternalInput")
    wk4 = nc.dram_tensor("wk4", [128, 4, KCHUNK, 128], dt.bfloat16,
                         kind="ExternalInput")
    wv = nc.dram_tensor("wv", [128, KCHUNK, DG], dt.bfloat16, kind="ExternalInput")
    wo = nc.dram_tensor("wo", [128, 4, D], dt.bfloat16, kind="ExternalInput")
    biasT = nc.dram_tensor("biasT", [128, KTILES], dt.float32, kind="ExternalInput")
    # partition-major output: out[p, qt, d] = partial[qt*128 + p, d]
    out = nc.dram_tensor("out", [128, QT, D], dt.bfloat16, kind="ExternalOutput")

    with tile.TileContext(nc) as tc, ExitStack() as ctx:
        const = ctx.enter_context(tc.tile_pool(name="const", bufs=1))

        mem_tiles = [
            const.tile([128, KCHUNK, 512], dt.bfloat16, name=f"mem_{c}")
            for c in range(NCH)
        ]
        kT = const.tile([128, 4, KT], dt.bfloat16)             # K^T  (dout, ktok)
        qT = const.tile([128, 4, Q], dt.bfloat16)              # Q^T  (dout, q)
        early_cm = tc.tile_pool(name="early", bufs=1)
        early = early_cm.__enter__()
        qry_q = [early.tile([128, 2, Q], dt.bfloat16, name=f"qry_{i}")
                 for i in range(4)]
        wq_mt = [early.tile([128, KCHUNK, 128], dt.bfloat16, name=f"wqm_{i}")
                 for i in range(4)]
        wv_h = [const.tile([128, 4, DG], dt.bfloat16, name=f"wv_{i}")
                for i in range(2)]
        wk_m0 = const.tile([128, KCHUNK, 128], dt.bfloat16)
        wk_rest = const.tile([128, 3, KCHUNK, 128], dt.bfloat16)
        wo_s = const.tile([128, 4, D], dt.bfloat16)
        bias_s = const.tile([128, KTILES], dt.float32)
        v_tiles = [
            const.tile([128, H_PER_CORE * 65], dt.bfloat16, name=f"v_{k}")
            for k in range(KTILES)
        ]
        att_pairs = [
            const.tile([128, Q], dt.bfloat16, name=f"attp_{p}") for p in range(4)
        ]
        warm_w = const.tile([128, 128], dt.bfloat16)
        warm_x = const.tile([128, 512], dt.bfloat16)
        nc.vector.memset(warm_w, 0.0)
        nc.vector.memset(warm_x, 0.0)
        warm_bc_in = const.tile([1, 64], dt.float32)
        warm_bc = const.tile([64, 64], dt.float32)

        # ---- DMA queue plan: flat slices, global need order, 3 queues.
        # scalar must clear before the first exp; sync/gpsimd split the
        # mem chunks in kt halves (kt 0-3 | kt 4-7).
        flat = lambda ap: ap.rearrange("p a b -> p (a b)")
        mem_f = lambda c: memT.ap()[:, c]                     # [128, 4096]
        mem_t = lambda c: mem_tiles[c].rearrange("p k t -> p (k t)")

        # scalar (slow start: ACT_TABLE_LOAD precedes its first trigger, so
        # nothing first-needed goes here): wq mt2, bias, wv half1
        nc.scalar.dma_start(wq_mt[2], wq4.ap()[:, 2])
        nc.scalar.dma_start(bias_s, biasT.ap())
        nc.scalar.dma_start(wv_h[1], wv.ap()[:, 4:8])
        # sync: qry/wq in consumption order, quarter-size pieces (each
        # trigger costs ~0.6us serial descriptor-gen on the queue)
        nc.sync.dma_start(qry_q[0], qryT.ap()[:, 0:2])
        nc.sync.dma_start(wq_mt[0], wq4.ap()[:, 0])
        nc.sync.dma_start(qry_q[1], qryT.ap()[:, 2:4])
        nc.sync.dma_start(wq_mt[3], wq4.ap()[:, 3])
        nc.sync.dma_start(mem_t(0)[:, 0:2048], mem_f(0)[:, 0:2048])
        nc.sync.dma_start(wv_h[0], wv.ap()[:, 0:4])
        nc.sync.dma_start(mem_t(1)[:, 2048:4096], mem_f(1)[:, 2048:4096])
        for c in (2, 3, 4, 5, 6, 7):
            nc.sync.dma_start(mem_t(c)[:, 0:2048], mem_f(c)[:, 0:2048])
        nc.sync.dma_start(wk_rest.rearrange("p m k j -> p (m k j)"),
                          wk4.ap()[:, 1:4].rearrange("p m k j -> p (m k j)"))
        # gpsimd: qry q2/q3, wq mt1, wk_m0, mem0 kt4-7, mem1a, ..., wo
        nc.gpsimd.dma_start(qry_q[2], qryT.ap()[:, 4:6])
        nc.gpsimd.dma_start(wq_mt[1], wq4.ap()[:, 1])
        nc.gpsimd.dma_start(qry_q[3], qryT.ap()[:, 6:8])
        nc.gpsimd.dma_start(wk_m0, wk4.ap()[:, 0])
        nc.gpsimd.dma_start(mem_t(0)[:, 2048:4096], mem_f(0)[:, 2048:4096])
        nc.gpsimd.dma_start(mem_t(1)[:, 0:2048], mem_f(1)[:, 0:2048])
        for c in (2, 3, 4, 5, 6, 7):
            nc.gpsimd.dma_start(mem_t(c)[:, 2048:4096], mem_f(c)[:, 2048:4096])
        nc.gpsimd.dma_start(wo_s.rearrange("p a b -> p (a b)"),
                            flat(wo.ap()))
        # warm the gpsimd partition_broadcast path (ucode library load +
        # first-use latency is ~8us; pair 0's normalize hits it otherwise).
        # Emitted AFTER the gpsimd dma triggers so the library load doesn't
        # delay SWDGE descriptor generation for the critical input fill.
        nc.vector.memset(warm_bc_in, 1.0)
        nc.gpsimd.partition_broadcast(warm_bc, warm_bc_in)

        for k in range(KTILES):
            vb = v_tiles[k].rearrange("p (h c) -> p h c", c=65)
            nc.vector.memset(vb[:, :, 64:65], 1.0)

        ps_stack = ExitStack()

        # HAM warmup: dependency-free matmuls keep the PE busy while DMAs
        # fill SBUF so the clock-gate is at full rate when real work starts.
        with tc.tile_pool(name="warm_ps", bufs=1, space="PSUM") as warm_ps:
            wps = warm_ps.tile([128, 512], dt.float32, tag="warm")
            for i in range(10):
                nc.tensor.matmul(wps, warm_w, warm_x,
                                 start=(i == 0), stop=(i == 9))

        proj_ps = ctx.enter_context(
            tc.tile_pool(name="proj_ps", bufs=2, space="PSUM")
        )

        kproj_state = {}

        def kproj_mms(mt, c, kts):
            """Emit K^T-projection matmuls for dout tile mt, chunk c, din
            tiles `kts`; evacuate after the last."""
            if mt == 0:
                wsrc, col = wk_m0, slice(None)
            else:
                wsrc, col = wk_rest[:, mt - 1], slice(None)
            if kts[0] == 0:
                kproj_state[(mt, c)] = proj_ps.tile(
                    [128, 512], dt.float32, tag="proj",
                    name=f"kproj_ps_{mt}_{c}")
            ps = kproj_state[(mt, c)]
            for kt in kts:
                nc.tensor.matmul(
                    ps, wsrc[:, kt, col], mem_tiles[c][:, kt, :],
                    start=(kt == 0), stop=(kt == KCHUNK - 1),
                )
            if kts[-1] == KCHUNK - 1:
                nc.vector.tensor_copy(kT[:, mt, ts(c, 512)], ps)
                del kproj_state[(mt, c)]

        def vproj(k):
            """V natural rows for ktile k (8 MMs + evac into v_tiles[k])."""
            c, j = k // 4, k % 4
            ps = proj_ps.tile([128, DG], dt.float32, tag="proj")
            for kt in range(KCHUNK):
                nc.tensor.matmul(
                    ps, mem_tiles[c][:, kt, ds(j * 128, 128)],
                    wv_h[kt // 4][:, kt % 4, :],
                    start=(kt == 0), stop=(kt == KCHUNK - 1),
                )
            vb = v_tiles[k].rearrange("p (h c) -> p h c", c=65)
            nc.vector.tensor_copy(
                vb[:, :, 0:64], ps.rearrange("p (h c) -> p h c", c=64)
            )

        # ---- Q projection: qT[dout, q] ----
        for mt in range(4):
            ps = proj_ps.tile([128, Q], dt.float32, tag="proj")
            for kt in range(KCHUNK):
                nc.tensor.matmul(
                    ps, wq_mt[mt][:, kt, :],
                    qry_q[kt // 2][:, kt % 2, :],
                    start=(kt == 0), stop=(kt == KCHUNK - 1),
                )
            nc.vector.tensor_copy(qT[:, mt, :], ps)
        early_cm.__exit__(None, None, None)

        # kT group (0,0) consumed at g=0; emit before the stream starts.
        kproj_mms(0, 0, list(range(KCHUNK)))

        # ---- flat attention stream over (pair, ktile) ----
        scores_ps = ps_stack.enter_context(
            tc.tile_pool(name="scores_ps", bufs=2, space="PSUM"))
        av_ps = ps_stack.enter_context(
            tc.tile_pool(name="av_ps", bufs=1, space="PSUM"))
        # bufs=5: with 3, exp(g) waits on AV matmuls of g-3 (+sem hop) and
        # the exp stream idles ~200ns/position in pairs 1-3
        p_pool = ps_stack.enter_context(tc.tile_pool(name="p_pool", bufs=5))
        norm_pool = ps_stack.enter_context(tc.tile_pool(name="norm_pool", bufs=1))

        avs = {}
        pending = []
        LAG = 2
        wo_groups = []

        def flush_one():
            pair, k, p_sb = pending.pop(0)
            if k == 0:
                avs[pair] = (
                    av_ps.tile([65, Q], dt.float32, tag="av0", name=f"av0_{pair}"),
                    av_ps.tile([65, Q], dt.float32, tag="av1", name=f"av1_{pair}"),
                )
            av = avs[pair]
            vb = v_tiles[k].rearrange("p (h c) -> p h c", c=65)
            for par, h in ((0, 2 * pair), (1, 2 * pair + 1)):
                nc.tensor.matmul(
                    av[par], vb[:, h, :], p_sb[:, par * Q:(par + 1) * Q],
                    start=(k == 0), stop=(k == KTILES - 1),
                )
            if k == KTILES - 1:
                pend = norm_phase1(pair)
                if pair == 3:
                    norm_phase2(pair, *pend)
                else:
                    norm_pend[pair] = pend

        def norm_phase1(pair):
            """Evacuate av psum + start the denominator hops.

            ALL four stage copies are emitted first so the av psum releases
            after ~1us of DVE work.  The denominator row (partition 64) is
            DMA-hopped to partition 0 for partition_broadcast (split across
            two queues: sync+gpsimd mid-stream, scalar+sync for pair 3).
            """
            av = avs[pair]
            den_eng = {1: nc.scalar if pair == 3 else nc.sync,
                       0: nc.sync if pair == 3 else nc.gpsimd}
            # the tiny denominator-row copies go first (so the den hops and
            # the reciprocal chain start ~1us earlier), then the two big
            # head copies; DMA can't read psum directly.
            stages = {}
            for par, h in ((1, 2 * pair + 1), (0, 2 * pair)):
                stage = norm_pool.tile([65, Q], dt.float32, tag="stage", bufs=2,
                                       name=f"stage_{h}")
                nc.vector.tensor_copy(stage[64:65, :], av[par][64:65, :])
                stages[par] = stage
            dens = {}
            for par, h in ((1, 2 * pair + 1), (0, 2 * pair)):
                den = norm_pool.tile([1, Q], dt.float32, tag="den", bufs=2,
                                     name=f"den_{h}")
                den_eng[par].dma_start(den, stages[par][64:65, :])
                dens[par] = den
            for par in (1, 0):
                nc.vector.tensor_copy(stages[par][0:64, :], avs[pair][par][0:64, :])
            return stages, dens

        def norm_phase2(pair, stages, dens):
            """Reciprocal + broadcast + normalize muls + odd-head hop.
            Deferred a few positions after phase 1 (for pairs 0-2) so the
            muls don't block kT evacuations in the DVE FIFO at the pair
            boundary; the odd head's normalized rows are DMA-hopped to
            partitions 64-127."""
            recvs = {}
            for par, h in ((1, 2 * pair + 1), (0, 2 * pair)):
                recv = norm_pool.tile([1, Q], dt.float32, tag="recv", bufs=2,
                                      name=f"recv_{h}")
                nc.vector.reciprocal_approx_fast(out=recv, in_=dens[par])
                recvs[par] = recv
            for par, h in ((1, 2 * pair + 1), (0, 2 * pair)):
                rec_bc = norm_pool.tile([64, Q], dt.float32, tag="rec_bc", bufs=2,
                                        name=f"rec_bc_{h}")
                nc.gpsimd.partition_broadcast(rec_bc, recvs[par])
                if par == 0:
                    nc.vector.tensor_mul(att_pairs[pair][0:64, :],
                                         stages[par][0:64, :], rec_bc)
                else:
                    odd = norm_pool.tile([64, Q], dt.bfloat16, tag="odd", bufs=2,
                                         name=f"odd_{h}")
                    nc.vector.tensor_mul(odd, stages[par][0:64, :], rec_bc)
                    eng = nc.scalar if pair == 3 else nc.sync
                    eng.dma_start(att_pairs[pair][64:128, :], odd)

        norm_pend = {}

        def wo_phase1(ps_pool, qt_i, dt_i, tag=None):
            ps = ps_pool.tile([128, 512], mybir.dt.float32,
                              tag=tag or f"wo_{qt_i}_{dt_i}",
                              name=f"wo_ps_{qt_i}_{dt_i}")
            for p in range(3):
                nc.tensor.matmul(
                    ps, att_pairs[p][:, ts(qt_i, 128)], wo_s[:, p, ts(dt_i, 512)],
                    start=(p == 0), stop=False,
                )
            wo_groups.append((qt_i, dt_i, ps))

        def scores(g):
            """Row-packed score pair for flat position g: even head on array
            rows 0-63, odd on 64-127 (concurrent K=64 matmuls)."""
            pair, k = g // 32, g % 32
            sc = scores_ps.tile([128, 2 * Q], dt.float32, tag="sc",
                                name=f"sc_{g}")
            nc.tensor.matmul(
                sc[:, 0:Q], kT[0:64, pair, ts(k, 128)], qT[0:64, pair, :],
                start=True, stop=True,
            )
            nc.tensor.matmul(
                sc[:, Q:2 * Q], kT[64:128, pair, ts(k, 128)], qT[64:128, pair, :],
                start=True, stop=True,
            )
            return sc

        # JIT production schedule: group n=1 at g=1,2 (4 MMs each); group
        # n>=2 spread 2 MMs/position over g=4n-6..4n-3 (consumed at g=4n).
        jit: dict[int, list] = {}
        jit.setdefault(1, []).append((0, 1, [0, 1, 2, 3]))
        jit.setdefault(2, []).append((0, 1, [4, 5, 6, 7]))
        for n in range(2, 32):
            for r in range(4):
                jit.setdefault(4 * n - 5 + r, []).append(
                    (n // 8, n % 8, [2 * r, 2 * r + 1]))

        sc_ahead = scores(0)
        for g in range(128):
            pair, k = g // 32, g % 32
            sc, sc_ahead = sc_ahead, (scores(g + 1) if g + 1 < 128 else None)
            p_sb = p_pool.tile([128, 2 * Q], dt.bfloat16, tag="p")
            nc.scalar.activation(
                p_sb, sc, mybir.ActivationFunctionType.Exp,
                bias=bias_s[:, k:k + 1], scale=SCALE,
            )
            pending.append((pair, k, p_sb))
            if pair == 0 and k >= 1:
                vproj(k - 1)
            elif g == 32:
                vproj(31)
            for mt, c, kts in jit.get(g, ()):
                kproj_mms(mt, c, kts)
            while len(pending) > LAG:
                flush_one()
            if g >= 36 and (g - 36) % 32 == 0 and (g - 36) // 32 in norm_pend:
                norm_phase2((g - 36) // 32, *norm_pend.pop((g - 36) // 32))
            # the K-proj JIT stream (last group done at g=121) frees proj_ps:
            # start two Wo psum groups in its banks
            if g == 122:
                wo_phase1(proj_ps, 0, 0, tag="proj")
            if g == 124:
                wo_phase1(proj_ps, 0, 1, tag="proj")
        while pending:
            flush_one()
        ps_stack.close()

        # remaining Wo groups: phase 1 (pairs 0-2) stationary-outer while
        # pair 3's normalize chain completes; phase 2 adds pair 3, casts
        # drain psum on two engines, two flat partition-major stores.
        with tc.tile_pool(name="wo_ps", bufs=1, space="PSUM") as wo_ps, \
                tc.tile_pool(name="out_pool", bufs=1) as out_pool:
            o_all = out_pool.tile([128, QT, D], mybir.dt.bfloat16, tag="o")
            rest = [(qt_i, dt_i) for qt_i in range(1, 4) for dt_i in range(2)]
            ps_map = {}
            for qt_i, dt_i in rest:
                ps_map[(qt_i, dt_i)] = wo_ps.tile(
                    [128, 512], mybir.dt.float32, tag=f"wo_{qt_i}_{dt_i}",
                    name=f"wo_ps_{qt_i}_{dt_i}")
            for p in range(3):
                for qt_i, dt_i in rest:
                    nc.tensor.matmul(
                        ps_map[(qt_i, dt_i)], att_pairs[p][:, ts(qt_i, 128)],
                        wo_s[:, p, ts(dt_i, 512)],
                        start=(p == 0), stop=False,
                    )
            for qt_i, dt_i in rest:
                wo_groups.append((qt_i, dt_i, ps_map[(qt_i, dt_i)]))
            # phase 2: qt-outer so att_pairs[3] stationaries serve 2 MMs
            groups_by_qt = {}
            for qt_i, dt_i, ps in wo_groups:
                groups_by_qt.setdefault(qt_i, []).append((dt_i, ps))
            for qt_i in range(4):
                for dt_i, ps in sorted(groups_by_qt[qt_i]):
                    nc.tensor.matmul(
                        ps, att_pairs[3][:, ts(qt_i, 128)],
                        wo_s[:, 3, ts(dt_i, 512)],
                        start=False, stop=True,
                    )
                for dt_i, ps in sorted(groups_by_qt[qt_i]):
                    if dt_i == 0:
                        nc.vector.tensor_copy(
                            o_all[:, qt_i, ds(0, 512)], ps)
                    else:
                        nc.scalar.copy(o_all[:, qt_i, ds(512, 512)], ps)
                if qt_i == 1:
                    nc.scalar.dma_start(
                        out.ap()[:, 0:2].rearrange("p a b -> p (a b)"),
                        o_all[:, 0:2].rearrange("p a b -> p (a b)"))
                if qt_i == 3:
                    nc.sync.dma_start(
                        out.ap()[:, 2:4].rearrange("p a b -> p (a b)"),
                        o_all[:, 2:4].rearrange("p a b -> p (a b)"))

    nc.compile()
    return nc


def _shard_inputs(query, memory, bias, Wq, Wk, Wv, Wo):
    """Host-side sharding + layout packing (per-core input dicts)."""
    in_maps = []
    packed = {}
    for hg in range(2):
        cols = slice(hg * DG, (hg + 1) * DG)
        # dout-tile-major packing: w4[p, mt, kt, j] = W[kt*128+p, mt*128+j]
        def mt_major(W):
            return np.ascontiguousarray(
                W[:, cols].reshape(KCHUNK, 128, 4, 128).transpose(1, 2, 0, 3)
            ).astype(BF16)
        packed[hg] = {
            "wq4": mt_major(Wq),
            "wk4": mt_major(Wk),
            "wv": np.ascontiguousarray(
                Wv[:, cols].reshape(KCHUNK, 128, DG).transpose(1, 0, 2)
            ).astype(BF16),
            # pair-stacked Wo: rows 0-63 = even head, 64-127 = odd head
            "wo": np.ascontiguousarray(
                Wo[cols, :].reshape(4, 128, D).transpose(1, 0, 2)
            ).astype(BF16),
        }
    for core in range(N_CORES):
        b, hg = core // 2, core % 2
        # memT[p, c, kt, t'] = mem[c*512 + t', kt*128 + p]
        m = memory[b].reshape(NCH, 512, KCHUNK, 128)        # (c, t', kt, p)
        memT_np = np.ascontiguousarray(m.transpose(3, 0, 2, 1)).astype(BF16)
        # qryT[p, kt, t] = query[t, kt*128 + p]
        qv = query[b].reshape(Q, KCHUNK, 128)               # (t, kt, p)
        qryT_np = np.ascontiguousarray(qv.transpose(2, 1, 0)).astype(BF16)
        in_maps.append(
            {
                "memT": memT_np.reshape(128, NCH, KCHUNK * 512),
                "qryT": qryT_np,
                "biasT": np.ascontiguousarray(bias[b].reshape(KTILES, 128).T).astype(F32),
                **packed[hg],
            }
        )
    return in_maps


def _get_nc():
    if "nc" not in _CACHE:
        _CACHE["nc"] = _build_nc()
    return _CACHE["nc"]


def run_sharded(inputs: dict, **run_kwargs):
    """Shard, run on 8 cores, gather. Returns (output, BassKernelResults)."""
    nc = _get_nc()
    in_maps = _shard_inputs(
        inputs["query"], inputs["memory"], inputs["bias"],
        inputs["Wq"], inputs["Wk"], inputs["Wv"], inputs["Wo"],
    )
    res = run_bass_kernel_spmd(nc, in_maps, core_ids=list(range(N_CORES)), **run_kwargs)
    out = np.empty((B, Q, D), dtype=F32)
    for b in range(B):
        # device output is partition-major [128, QT, D]
        o0 = res.results[2 * b]["out"].astype(F32)
        o1 = res.results[2 * b + 1]["out"].astype(F32)
        out[b] = (o0 + o1).transpose(1, 0, 2).reshape(Q, D)
    return out, res


def kernel(**inputs) -> np.ndarray:
    inputs = {k: np.asarray(v) for k, v in inputs.items()}
    out, _ = run_sharded(inputs)
    return out
